# revision 51
# baseline (speedup 1.0000x reference)
"""AdEx E/I recurrent-network single-step kernel for 8 Trainium2 NeuronCores.

v5 strategy: tensor-parallel column-shard over UNITS (no collectives);
each core computes its 512 output columns of i_t = x@Wi + z@Wr from the
full activations plus a 512-column shard of both weight matrices, and the
HOST does every elementwise step (AdEx voltage/adaptation/spike/refractory
chains are all batch-local functions of the step inputs plus i_t).

The kernel is a pure GEMM pipeline, bound by the serialized DMA stream
(~360 GB/s, one shared DMA-engine resource) and the PE (~10.3us of fp8
DoubleRow matmuls at 2.4GHz):
  - Weights-STATIONARY orientation: lhsT = weight block [128k, 128 units],
    rhs = activations [128k, 512 batch], PSUM = [128 units, 512 batch].
    Each 128-unit block's accumulation stops right after its own slice of
    the weight stream lands, so PSUM drains + output DMAs overlap the
    remaining stream instead of serializing after it.
  - Binary z ships as PACKED BITS (k-major) and is expanded on-chip into
    the recurrent GEMM's fp8 rhs by 16 u16 tensor_scalar shift+and ops
    (DVE, split by k-half so the PE starts ~1us earlier). The expansion
    writes the exact fp8e4m3 byte 0x08 (= 1/64); the x64 recurrent-weight
    prescale compensates. Bit b of packed byte j expands to batch position
    b*64+j, imposing the permutation PERM[p] = 8*(p%64)+p//64 on the
    batch axis; the host permutes x rows to match and un-permutes outputs.
  - The PE p-state ramp (full clock only after 3us of continuous busy)
    is absorbed by a chain of dummy matmuls on memset data issued before
    the real work; idle gaps reset the ramp, so the chain ends just as
    the first real operands land.
  - Stream order = PE consumption order: zpk, recurrent weights u0..u3,
    input weights / xt interleaved, with the final chunks split small so
    each block's stop trails its last byte (+900ns DMA-sem latency) as
    little as possible.
  - Every PSUM drains as two parallel fp16 halves (ACT + DVE) into
    per-engine SBUF tiles; the last block accumulates into two batch-half
    PSUM tiles with distinct stop counts so both its drains wait their
    own PE stop directly (Tile's sem assignment otherwise chains the
    DVE drain behind the ACT drain, ~+0.6us).
  - Four output DMAs (one per engine-half x block-pair) split across the
    SP and ACT queues, issued after all input DMAs so the in-order
    queues never block the input stream.
  - Keep the PE instruction count under ~128: above that the Tile
    scheduler coarsens cross-engine waits (observed at 142-152: drains
    collapse to waiting for ALL matmuls).

Host assembly: i_t (fp16, abs err ~0.05 -> ~2e-4 on new_v) -> exact f32
reference math for new_v/new_z/new_w/new_r (Dale's-law constraint is a
no-op; only the autapse diagonal mask matters, folded into the shipped
weights).
"""

import ml_dtypes
import numpy as np

from concourse import bacc
import concourse.mybir as mybir
from concourse.bass_utils import run_bass_kernel_spmd
from concourse.tile import TileContext

B, N_IN, UNITS, CORES = 512, 2048, 4096, 8
US = UNITS // CORES          # 512 units per core
NB = US // 128               # 4 unit blocks per core
KI = N_IN // 128             # 16 k-tiles, input GEMM
KR = UNITS // 128            # 32 k-tiles, recurrent GEMM
KB = KR + KI                 # 48 k-tiles of weights per unit block

DT = 1.0; GL = 30.0; CAP = 281.0; EL = -70.6; THR = -50.4; DELTAT = 2.0
TAUW = 144.0; A_W = 4.0; B_W = 0.0805; V_RESET = -70.6; N_REFRAC = 2
DT_GL_C = DT * GL / CAP
DT_A_TAUW = DT * A_W / TAUW

FP8_DT = mybir.dt.float8e4
FP8_NP = mybir.dt.np(mybir.dt.float8e4)
F16 = mybir.dt.float16
U8 = mybir.dt.uint8
U16 = mybir.dt.uint16
F32 = mybir.dt.float32
WR_PRESCALE = 64.0            # recurrent weights x64; z expands to 1/64

# Expansion-imposed batch permutation: position p holds batch PERM[p].
PERM = np.array([8 * (p % 64) + p // 64 for p in range(512)], dtype=np.int64)

LAST_RESULTS = None
TRACE = False
N_WARM = 8         # f32 ramp matmuls (+1 bf16); ends as real data arrives


def _build_nc():
    nc = bacc.Bacc("TRN2", target_bir_lowering=False)

    # Strip init-time const-AP memsets + all-engine barrier (unused here).
    _b0 = nc.m.functions[0].blocks[0]
    _b0.instructions = [
        i for i in _b0.instructions
        if type(i).__name__ not in ("InstMemset", "InstDrain", "InstEventSemaphore")
    ]

    zpk_in = nc.declare_dram_parameter("zpk", [128, KR * 64], U8, isOutput=False)
    xt_in = nc.declare_dram_parameter("xt", [128, KI * 512], FP8_DT,
                                      isOutput=False)
    wb_in = nc.declare_dram_parameter("wb", [128, NB * KB * 128], FP8_DT,
                                      isOutput=False)
    it_out = nc.declare_dram_parameter("it", [128, NB * 512], F16,
                                       isOutput=True)

    AF = mybir.ActivationFunctionType
    OP = mybir.AluOpType

    with TileContext(nc) as tc:
        with (
            tc.tile_pool(name="gemm_in", bufs=1) as gpool,
            tc.tile_pool(name="outs", bufs=1) as opool,
            tc.tile_pool(name="psum", bufs=1, space="PSUM") as ppool,
        ):
            # NOTE: keep the total PE instruction count under ~128 — above
            # that the Tile scheduler coarsens cross-engine waits (observed:
            # drain waits collapse to "all matmuls done").
            # u0/u1 accumulate in one PSUM tile each; u2/u3 (whose drains
            # sit on the critical tail) accumulate in two batch-half PSUM
            # tiles with separate stop counts: each half-drain then waits
            # its own PE stop directly instead of being sem-chained behind
            # its sibling (ACT-before-DVE) drain.
            NSPLIT = 1
            ps = [ppool.tile([128, 512], F32, tag=f"ps{u}", name=f"ps{u}")
                  for u in range(NB - NSPLIT)]
            psx = {u: [ppool.tile([128, 256], F32, tag=f"psx{u}{h}",
                                  name=f"psx{u}{h}") for h in range(2)]
                   for u in range(NB - NSPLIT, NB)}
            warm_ps = ppool.tile([128, 512], F32, tag="warmps",
                                 name="warmps")

            zpk_t = gpool.tile([128, KR, 64], U8, tag="zpk", name="zpk")
            zx_t = gpool.tile([128, KR, B], FP8_DT, tag="zx", name="zx")
            xt_t = gpool.tile([128, KI, 512], FP8_DT, tag="xt", name="xt")
            wb_t = gpool.tile([128, NB * KB, 128], FP8_DT, tag="wb",
                              name="wb")
            # Separate drain tiles per engine: if ACT and DVE both write
            # one tile, Tile's WAW tracking chains the DVE drains behind
            # the ACT ones (~+0.6us on the tail).
            ita_t = opool.tile([128, NB * 256], F16, tag="ita", name="ita")
            itb_t = opool.tile([128, NB * 256], F16, tag="itb", name="itb")
            warm_t = gpool.tile([128, 2, 128], F32, tag="warm", name="warm")

            # PE p-state ramp: the tensor engine reaches full clock only
            # after 3us of CONTINUOUS busy (idle resets the clock). Run a
            # chain of dummy matmuls on Pool-memset data from t~0.2us until
            # real data arrives (~4.6us) so every real matmul runs at
            # 2.4GHz and the PE never idles before its real work. f32
            # matmuls cost 4 cycles/row, so few instructions cover the
            # window — keeping the PE instruction count under the wait-
            # coarsening threshold (~128).
            nc.gpsimd.memset(warm_t, 0.0)
            wbf = warm_t.bitcast(mybir.dt.bfloat16)  # [128, 2, 256] bf16
            for _ in range(N_WARM):
                nc.tensor.matmul(warm_ps[:, :128], lhsT=warm_t[:, 0, :],
                                 rhs=warm_t[:, 0, :], start=True, stop=True)
            nc.tensor.matmul(warm_ps[:, :256], lhsT=wbf[:, 0, :128],
                             rhs=wbf[:, 0, :], start=True, stop=True)

            def dma_wb(u, k0, n):
                """Stream k-tiles [k0, k0+n) of unit block u."""
                s = u * KB + k0
                nc.sync.dma_start(
                    out=wb_t[:, s:s + n, :],
                    in_=wb_in.ap()[:, s * 128:(s + n) * 128]
                    .rearrange("p (k m) -> p k m", k=n))

            def dma_xt(k0, n):
                nc.sync.dma_start(
                    out=xt_t[:, k0:k0 + n, :],
                    in_=xt_in.ap()[:, k0 * 512:(k0 + n) * 512]
                    .rearrange("p (k m) -> p k m", k=n))

            # --- DMA stream (SP queue, in PE consumption order):
            # zpk | recurrent weights u0..u3 | input weights interleaved
            # with xt halves, fine-grained near the end so each block's
            # stop trails its data as little as possible.
            nc.sync.dma_start(
                out=zpk_t,
                in_=zpk_in.ap().rearrange("p (k b) -> p k b", k=KR))
            dma_wb(0, 0, 16)         # u0 recurrent k0-15 (earliest PE work)
            dma_wb(0, 16, 16)        # u0 recurrent k16-31
            dma_wb(1, 0, KR)
            dma_wb(2, 0, KR)
            dma_wb(3, 0, KR)
            dma_wb(0, KR, KI)        # input weights u0
            dma_wb(3, KR, 8)         # input weights u3, first half
            dma_xt(0, 8)
            dma_xt(8, 8)
            dma_wb(1, KR, KI)
            dma_wb(2, KR, KI)
            dma_wb(3, KR + 8, 4)
            dma_wb(3, KR + 12, 4)

            # z expansion: u16 shift+and ops, one per (bit, k-half), each
            # writing contiguous 64-byte blocks per k-tile. Split by k-half
            # so the first recurrent matmuls start ~1.1us earlier.
            zpk16 = zpk_t.bitcast(U16)            # [128, KR, 32]
            zx16 = zx_t.bitcast(U16)              # [128, KR, 256]
            for half in range(2):
                ks = slice(16 * half, 16 * (half + 1))
                for b in range(8):
                    dst = zx16[:, ks, b * 32:(b + 1) * 32]
                    if b >= 3:
                        nc.vector.tensor_scalar(
                            out=dst, in0=zpk16[:, ks, :], scalar1=b - 3,
                            scalar2=0x0808,
                            op0=OP.logical_shift_right, op1=OP.bitwise_and)
                    else:
                        nc.vector.tensor_scalar(
                            out=dst, in0=zpk16[:, ks, :], scalar1=3 - b,
                            scalar2=0x0808,
                            op0=OP.logical_shift_left, op1=OP.bitwise_and)

            # --- Matmuls, fp8 DoubleRow.
            def mm(u, k0, rhs_t, rk0, start, stop):
                lhsT = wb_t[:, u * KB + k0:u * KB + k0 + 2, :]
                if u in psx:
                    for h in range(2):
                        nc.tensor.matmul(
                            psx[u][h], lhsT=lhsT,
                            rhs=rhs_t[:, rk0:rk0 + 2,
                                      h * 256:(h + 1) * 256],
                            start=start, stop=stop,
                            perf_mode=mybir.MatmulPerfMode.DoubleRow)
                else:
                    nc.tensor.matmul(
                        ps[u], lhsT=lhsT, rhs=rhs_t[:, rk0:rk0 + 2, :],
                        start=start, stop=stop,
                        perf_mode=mybir.MatmulPerfMode.DoubleRow)

            def rec_phase(u, half=None):
                kps = (range(0, KR, 2) if half is None else
                       range(16 * half, 16 * half + 16, 2))
                for kp in kps:
                    mm(u, kp, zx_t, kp, kp == 0, False)

            def in_phase(u, half, stop=False):
                for kp in range(8 * half, 8 * half + 8, 2):
                    mm(u, KR + kp, xt_t, kp, False,
                       stop and kp == 8 * half + 6)

            # Drain a block's PSUM to fp16 as two parallel halves
            # (ACT + DVE) so each output's DMA launches ~0.4us after its
            # stop. Issued INLINE right after the block's stop matmul —
            # end-of-program batching makes Tile coarsen the DVE waits.
            def drain(u):
                s = slice(u * 256, (u + 1) * 256)
                if u in psx:
                    plo, phi = psx[u][0], psx[u][1]
                else:
                    plo, phi = ps[u][:, :256], ps[u][:, 256:]
                nc.scalar.activation(ita_t[:, s], plo, AF.Copy)
                nc.vector.tensor_scalar(
                    out=itb_t[:, s], in0=phi, scalar1=1.0,
                    scalar2=None, op0=OP.mult, op1=OP.bypass)

            # All recurrent phases first (zx is on-chip; weights stream at
            # 360B/ns vs PE's 306B/ns), u0/u1's k-halves interleaved so
            # the PE has k0-15 work while the DVE expands k16-31; then the
            # input sub-phases ordered by data arrival, stops staggered.
            for u in range(NB):
                rec_phase(u)
            in_phase(0, 0)
            in_phase(3, 0)
            in_phase(0, 1, stop=True)
            drain(0)
            in_phase(1, 0)
            in_phase(1, 1, stop=True)
            drain(1)
            in_phase(2, 0)
            in_phase(2, 1, stop=True)
            drain(2)
            in_phase(3, 1, stop=True)
            drain(3)

            # Output DMAs per block pair and engine-half, split across the
            # SP and ACT queues so their launch overheads overlap. DRAM
            # layout: [u01-lo | u23-lo | u01-hi | u23-hi].
            oap = it_out.ap()
            nc.sync.dma_start(out=oap[:, 0:512], in_=ita_t[:, 0:512])
            nc.scalar.dma_start(out=oap[:, 1024:1536], in_=itb_t[:, 0:512])
            nc.sync.dma_start(out=oap[:, 512:1024], in_=ita_t[:, 512:1024])
            nc.scalar.dma_start(out=oap[:, 1536:2048],
                                in_=itb_t[:, 512:1024])

    nc.compile()
    return nc


_NC_CACHE = {}


def _get_nc(binary_z=True):
    if "nc" not in _NC_CACHE:
        _NC_CACHE["nc"] = _build_nc()
    return _NC_CACHE["nc"]


def kernel(inputs, v, r, w, z, input_weights, recurrent_weights):
    inputs = np.asarray(inputs, dtype=np.float32)
    v = np.asarray(v, dtype=np.float32)
    r = np.asarray(r)
    w = np.asarray(w, dtype=np.float32)
    z = np.asarray(z, dtype=np.float32)
    input_weights = np.asarray(input_weights, dtype=np.float32)
    recurrent_weights = np.asarray(recurrent_weights, dtype=np.float32)

    wrec = recurrent_weights.copy()
    np.fill_diagonal(wrec, 0.0)
    # Dale's law constraint sign(w)*w_masked >= 0 is identically true.

    binary_z = bool(np.all((z == 0.0) | (z == 1.0)))
    i_t = _gemm_on_hw(inputs, z, input_weights, wrec) if binary_z else None
    if i_t is None or not np.isfinite(i_t).all():
        # Fallback: exact host GEMM (non-binary z or bad HW result).
        i_t = inputs @ input_weights + z @ wrec

    return _assemble(i_t, v, r, w, z)


def _assemble(i_t, v, r, w, z):
    r_dtype = r.dtype
    ri = r.astype(np.int32)
    exp_terms = np.clip(
        np.exp((v - THR) / DELTAT), -1.0e6, 30.0 / DT_GL_C).astype(np.float32)
    new_v = (v - DT_GL_C * (v - EL) + DT_GL_C * DELTAT * exp_terms
             + (i_t - w) * (DT / CAP)).astype(np.float32)
    new_v = np.where(z > 0.5, np.float32(V_RESET), new_v)
    new_w = (w - DT / TAUW * w + DT_A_TAUW * (v - EL)
             + B_W * z).astype(np.float32)
    v_scaled = -(THR - new_v) / (THR - EL)
    new_z = (v_scaled > 0.0).astype(np.float32)
    new_z = np.where(ri > 0, np.float32(0.0), new_z)
    new_r = np.clip(ri - 1 + (new_z * N_REFRAC).astype(np.int32), 0, N_REFRAC)
    return (np.ascontiguousarray(new_v), new_z,
            np.ascontiguousarray(new_w), new_r.astype(r_dtype))


def _gemm_on_hw(inputs, z, input_weights, wrec):
    """i_t = inputs @ input_weights + z @ wrec on the 8 NeuronCores."""
    global LAST_RESULTS
    FP8_MAX = np.float32(240.0)

    def to_fp8(a):
        return np.clip(a, -FP8_MAX, FP8_MAX).astype(FP8_NP)

    x_p = inputs[PERM]
    xT = np.ascontiguousarray(to_fp8(x_p).T)         # [2048, 512]
    xt = np.ascontiguousarray(
        xT.reshape(KI, 128, 512).transpose(1, 0, 2).reshape(128, KI * 512))

    wi_s = to_fp8(input_weights)                     # [2048, 4096]
    wr_s = to_fp8(wrec * np.float32(WR_PRESCALE))    # [4096, 4096]

    zpk = np.packbits(z.T.astype(np.uint8), axis=1, bitorder="little")
    zpk = np.ascontiguousarray(
        zpk.reshape(KR, 128, 64).transpose(1, 0, 2).reshape(128, KR * 64))

    in_maps = []
    for c in range(CORES):
        cs = slice(c * US, (c + 1) * US)
        # Per unit block: recurrent k-tiles then input k-tiles, each
        # [128 kpart, kt, 128 units] flattened k-major per partition.
        wr_c = wr_s[:, cs].reshape(KR, 128, NB, 128)     # [kt, p, u, m]
        wi_c = wi_s[:, cs].reshape(KI, 128, NB, 128)
        wb = np.concatenate([
            wr_c.transpose(1, 2, 0, 3),                  # [p, u, KR, 128]
            wi_c.transpose(1, 2, 0, 3),                  # [p, u, KI, 128]
        ], axis=2)                                       # [p, u, KB, 128]
        wb = np.ascontiguousarray(wb.reshape(128, NB * KB * 128))
        in_maps.append({"zpk": zpk, "xt": xt, "wb": wb})

    nc = _get_nc()
    res = run_bass_kernel_spmd(nc, in_maps, core_ids=list(range(CORES)),
                               trace=TRACE)
    LAST_RESULTS = res

    # it[c] is [128, NB*512] fp16 laid out [u01-lo|u23-lo|u01-hi|u23-hi]:
    # partition p, block u, batch col j -> unit c*512+u*128+p, batch
    # PERM[j], with j 0-255 in the lo half and 256-511 in the hi half.
    def unpack(arr):
        a = np.asarray(arr).astype(np.float32)
        lo = a[:, :NB * 256].reshape(128, NB, 256)
        hi = a[:, NB * 256:].reshape(128, NB, 256)
        return (np.concatenate([lo, hi], axis=2)
                .transpose(1, 0, 2).reshape(US, 512))

    blocks = [unpack(res.results[c]["it"]) for c in range(CORES)]
    it_perm = np.concatenate(blocks, axis=0).T       # [512 perm rows, 4096]
    inv = np.empty_like(PERM)
    inv[PERM] = np.arange(512)
    return np.ascontiguousarray(it_perm[inv])
